# revision 15
# baseline (speedup 1.0000x reference)
"""Trainium2 Bass kernel for nn_EpiNN_aaindex (pairwise-MLP GNN reduction).

Math (per batch b):
  x1 = emb@tw + tb                              (computed on host)
  X[i,d] = emb[i*64+d] * tw[i*64+d]             (L=256, D=64; on host)
  s_ij = MLP(concat[(x_i+x_j)/2, |x_i-x_j|])    (64->16->1, LeakyReLU 0.01)
  out_b = x1 + scale * sum_{i<j} s_ij

Strategy: 8 cores, 4 batches/core (data parallel over B=32).
Exact upper-triangle enumeration via cyclic offsets o=1..128:
pairs (i, (i+o) mod 256) for o=1..127 cover each unordered pair once;
o=128 covers each pair twice (0.5 correction on host).

Device inputs per batch (host-precomputed, bf16):
  XU  [128, 512]: both partition halves = [X2T | X2T]  (X2T = X.T [64, 256])
  XSo [128, 320]: top = X2T shifted 1, bottom = shifted 65  (odd offsets)
  XSe [128, 320]: top = X2T shifted 2, bottom = shifted 66  (even offsets)

Main loop: 16 iterations per batch, 8 offsets each, software-pipelined
with a 1-iteration skew so the scalar (ACT) engine never waits on the
H1 -> P2 -> HJ chain:
  iteration u emits: A2(u+1) [DVE sub+abs], P1(u+1) [16 quadrant MMs],
  H1(u+1) [ACT Lrelu], P2(u) [4 col-tiled MMs, dense 8x16 packing],
  HJ [ACT Lrelu+accum, one per 2 iterations, FD=512].

P1 quadrants (64x64 tile mode, 4 concurrent): 4 MMs each =
  w1b@|xi-xj| + 0.5*w1a@xi (XU) + 0.5*w1a@xj (XSx windows).
P2 dense: W2D [128, 32] block-diag(w2.T, w2.T); col-tile j reads
  H1[:, 256j:256j+256], writes P2[32j:32j+32] -> all 128 partitions
  useful (8 offsets x 16 basis), so HJ activation FD is 4x smaller.

Final combine on host: out = x1 + scale*(w3 . R + 32640*b3).
"""
import numpy as np

L, D = 256, 64
B_PER_CORE = 4
N_CORES = 8
NPAIRS = 32640  # 256*255/2
N_ITERS = 16
# quads 0-2 -> cols 0-2; quad 3: its 12-14 -> col 3, it15 even cols -> col 4,
# it15 odd cols (incl. offset o=128) -> col 5
ACC_COLS = 6

_CACHE = {}
import os as _os
N_RUN_CORES = int(_os.environ.get("EPINN_CORES", str(N_CORES)))
HJ_MODE = _os.environ.get("EPINN_HJ", "act")  # act | dve | alt


def _register_custom_ops():
    """Register fused DVE ops via the dve_ops extension point (per-NEFF
    uop table; sha pins harvested at first compile)."""
    import re
    import numpy as np
    from concourse import dve_ops
    from concourse.dve_spec import Spec, Src0, Src1, C0, C1, Zero, maxx
    from operator import add

    if "ops" in _CACHE:
        return _CACHE["ops"]
    existing = {o.name for o in dve_ops.OPS}

    def ref_sub_abs(in0, in1, s0, s1, imm2):
        return np.abs(in0.astype(np.float32) - in1.astype(np.float32))

    def ref_lrelu_acc(in0, in1, s0, s1, imm2):
        z = in0.astype(np.float32) + s0
        b = np.maximum(z, z * s1)
        return b, b.reshape(b.shape[0], -1).sum(axis=-1, keepdims=True)

    specs = {
        "ANT_SUB_ABS": (Spec(body=maxx(Src0 - Src1, Src1 - Src0),
                             reference=ref_sub_abs), {"v3": True, "v4": True}),
        "ANT_LRELU_ACC": (Spec(body=maxx(Src0 + C0, (Src0 + C0) * C1),
                               accum=add, accum_init=Zero,
                               reference=ref_lrelu_acc), {}),
    }
    out = {}
    for name, (spec, perf_en) in specs.items():
        if name in existing:
            out[name] = next(o for o in dve_ops.OPS if o.name == name)
            continue
        row = dve_ops._CUSTOM_DVE_ROW_BASE + len(dve_ops.OPS)
        dve_ops._SUB_OPCODE_FOR_NAME[name] = row
        sha = {}
        op = None
        for _ in range(4):
            op = dve_ops.DveOp(name, spec, False, dict(sha), perf_en)
            try:
                for ver in ("v3", "v4"):
                    op.compile(ver)
                break
            except ValueError as e:
                m = re.search(r"\((v\d+): ([0-9a-f]+)", str(e))
                if not m:
                    raise
                sha[m.group(1)] = m.group(2)
        dve_ops.OPS.append(op)
        out[name] = op
    _CACHE["ops"] = out
    return out


def _build_program():
    import concourse.bacc as bacc
    import concourse.mybir as mybir
    import concourse.tile as tile
    from contextlib import ExitStack

    f32 = mybir.dt.float32
    bf16 = mybir.dt.bfloat16
    u16 = mybir.dt.uint16
    AF = mybir.ActivationFunctionType
    ALU = mybir.AluOpType

    OPS = _register_custom_ops()
    nc = bacc.Bacc("TRN2", target_bir_lowering=False, debug=False,
                   num_devices=N_CORES)

    # ---- DRAM parameters (per core) ----
    xz_d = nc.declare_dram_parameter("xz", [B_PER_CORE, 128, 2320], bf16,
                                     isOutput=False)
    cw_d = nc.declare_dram_parameter("cw", [128, 160], bf16, isOutput=False)
    cb_d = nc.declare_dram_parameter("cb", [128, 2], f32, isOutput=False)

    acc_o = nc.declare_dram_parameter("acc_o", [B_PER_CORE, 128, ACC_COLS],
                                      f32, isOutput=True)

    with tile.TileContext(nc) as tc, ExitStack() as ctx:
        cpool = ctx.enter_context(tc.tile_pool(name="consts", bufs=1))
        xpool = ctx.enter_context(tc.tile_pool(name="xbufs", bufs=3))
        apool = ctx.enter_context(tc.tile_pool(name="abufs", bufs=4))
        hpool = ctx.enter_context(tc.tile_pool(name="hbufs", bufs=4))
        jpool = ctx.enter_context(tc.tile_pool(name="junk", bufs=3))
        opool = ctx.enter_context(tc.tile_pool(name="outs", bufs=2))
        pp1 = ctx.enter_context(tc.tile_pool(name="p1", bufs=2, space="PSUM"))
        pp2 = ctx.enter_context(tc.tile_pool(name="p2", bufs=2, space="PSUM"))
        fold_b2 = HJ_MODE in ("dve", "alt")

        # ---- static weights / consts (2 packed DMAs) ----
        CW = cpool.tile([128, 160], bf16)
        CB = cpool.tile([128, 2], f32)
        nc.sync.dma_start(CW[:], cw_d[:])
        nc.sync.dma_start(CB[:], cb_d[:])
        WB = CW[:, 0:64]
        WA = CW[:, 64:128]
        W2D = CW[:, 128:160]
        B1S = CB[:, 0:1]
        B2D = CB[:, 1:2]

        # ---- per-batch input tiles (loaded by DMA, double buffered) ----
        xt = {}

        def load_batch(b):
            XZ = xpool.tile([128, 2320], bf16, tag="xz")
            nc.sync.dma_start(XZ[:], xz_d[b])
            xt[b] = XZ

        load_batch(0)

        NB = B_PER_CORE
        NU = NB * N_ITERS

        def emit_front(u):
            """A2 build + P1 matmuls + H1 activation for global iter u."""
            b, it = divmod(u, N_ITERS)
            XZ = xt[b]
            XU = XZ[:, 0:512]
            XS = XZ[:, 1024:1672]          # XSi copy0
            XU2v = XZ[:, 0:1024].rearrange("p (g c) -> p g c", g=2, c=512)
            XS2v = XZ[:, 1024:2320].rearrange("p (g c) -> p g c", g=2, c=648)
            c0 = 4 * it

            A2 = apool.tile([128, 1024], bf16, tag="a2")
            # interleaved layout: col 512g + 2k + w = pair (i=k [+128 per
            # later j split], offset 4it+1+2g+w); partition halves: top =
            # base offset, bottom = +64.  XS col 2k+w = X2T[(k+1+w)%256]
            # (top) / X2T[(k+65+w)%256] (bottom); XU col 2k+w = X2T[k].
            A2v = A2[:, :].rearrange("p (g c) -> p g c", g=2, c=512)
            nc.vector._custom_dve(OPS["ANT_SUB_ABS"], out=A2v,
                                  in0=XU2v,
                                  in1=XS2v[:, :, 2 * c0:2 * c0 + 512])

            # P1: 4 concurrent 64x64 quadrants, 3 MMs each; psum col group
            # 0:512 <- window 2c0, 512:1024 <- window 2c0+4 (halves swapped).
            P1 = pp1.tile([128, 1024], f32, tag="p1")
            for (pc, tp, ar) in (
                (0, (0, 0), 0),        # T0
                (0, (64, 64), 64),     # T10
                (512, (64, 0), 64),    # T8  (data rows 64:128 -> psum 0:64)
                (512, (0, 64), 0),     # T2  (data rows 0:64 -> psum 64:128)
            ):
                rg, pr = ar, tp[1]
                cw = 2 * c0 if pc == 0 else 2 * c0 + 4
                ps = P1[pr:pr + 64, pc:pc + 512]
                nc.tensor.matmul(ps, WB[rg:rg + 64, :], A2[rg:rg + 64, pc:pc + 512],
                                 start=True, stop=False, tile_position=tp,
                                 skip_group_check=True)
                nc.tensor.matmul(ps, WA[rg:rg + 64, :], XU[rg:rg + 64, 0:512],
                                 start=False, stop=False, tile_position=tp,
                                 skip_group_check=True)
                nc.tensor.matmul(ps, WA[rg:rg + 64, :],
                                 XS[rg:rg + 64, cw:cw + 512],
                                 start=False, stop=True, tile_position=tp,
                                 skip_group_check=True)

            H1 = hpool.tile([128, 1024], bf16, tag="h1")
            nc.scalar.activation(H1[:], P1[:], AF.Lrelu, bias=B1S[:],
                                 scale=1.0, alpha=0.01)
            return H1

        # ---- software-pipelined main loop ----
        state = {"H1": None, "P2": None, "ACC": None}

        def emit_back(u):
            """P2 matmuls for iter u (+ layer-2 lrelu/sum when quad done)."""
            b, it = divmod(u, N_ITERS)
            if it == 0:
                ACC = opool.tile([128, ACC_COLS], f32, tag="acc", name="ACC")
                nc.gpsimd.memset(ACC[:], 0.0)
                state["ACC"] = ACC
            ACC = state["ACC"]

            ph = it % 4
            q = it // 4
            if ph == 0:
                state["P2"] = pp2.tile([128, 1024], f32, tag="p2", name="P2")
            P2 = state["P2"]
            H1 = state["H1"]
            for j in range(4):
                ps = P2[32 * j:32 * j + 32, 256 * ph:256 * ph + 256]
                nc.tensor.matmul(ps, W2D[:], H1[:, 256 * j:256 * j + 256],
                                 start=True, stop=True,
                                 tile_position=(0, 32 * j),
                                 skip_group_check=True)

            if ph == 3:
                HQ = jpool.tile([128, 1024], bf16, tag="hq", name="HQ")
                LR = OPS["ANT_LRELU_ACC"]
                if it == 15:
                    # o=128 lives in odd cols of phase-3 block (partitions
                    # 64:80 and 96:112) -> isolate in its own accum col
                    HQv = HQ[:, 768:1024].rearrange("p (k w) -> p k w", w=2)
                    P2v = P2[:, 768:1024].rearrange("p (k w) -> p k w", w=2)
                    nc.vector._custom_dve(LR, out=HQ[:, 0:768],
                                          in0=P2[:, 0:768], s0=B2D[:], s1=0.01,
                                          accum_out=ACC[:, 3:4])
                    nc.scalar.activation(HQv[:, :, 0], P2v[:, :, 0], AF.Lrelu,
                                         bias=B2D[:], scale=1.0, alpha=0.01,
                                         accum_out=ACC[:, 4:5])
                    nc.scalar.activation(HQv[:, :, 1], P2v[:, :, 1], AF.Lrelu,
                                         bias=B2D[:], scale=1.0, alpha=0.01,
                                         accum_out=ACC[:, 5:6])
                    nc.sync.dma_start(acc_o[b], ACC[:])
                elif q == 1:
                    nc.scalar.activation(HQ[:], P2[:], AF.Lrelu, bias=B2D[:],
                                         scale=1.0, alpha=0.01,
                                         accum_out=ACC[:, q:q + 1])
                else:
                    nc.vector._custom_dve(LR, out=HQ[:], in0=P2[:],
                                          s0=B2D[:], s1=0.01,
                                          accum_out=ACC[:, q:q + 1])

        state["H1"] = emit_front(0)
        for u in range(NU):
            b, it = divmod(u, N_ITERS)
            if it == 6 and b + 1 < NB:
                load_batch(b + 1)
            H1_next = emit_front(u + 1) if u + 1 < NU else None
            emit_back(u)
            state["H1"] = H1_next

    nc.compile()
    return nc


def _get_program():
    if "prog" not in _CACHE:
        _CACHE["prog"] = _build_program()
    return _CACHE["prog"]


def _get_runner():
    """Build (once) a cached jitted SPMD executable for the program."""
    key = ("runner", N_RUN_CORES)
    if key in _CACHE:
        return _CACHE[key]
    import jax
    import numpy as _np
    import concourse.mybir as mybir
    from jax.sharding import Mesh, PartitionSpec
    from jax.experimental.shard_map import shard_map
    from concourse import bass2jax
    from concourse.bass2jax import _bass_exec_p, partition_id_tensor

    bass2jax.install_neuronx_cc_hook()
    nc = _get_program()
    n_cores = N_RUN_CORES

    partition_name = (nc.partition_id_tensor.name
                      if nc.partition_id_tensor else None)
    in_names, out_names, out_avals, zero_shapes = [], [], [], []
    for alloc in nc.m.functions[0].allocations:
        if not isinstance(alloc, mybir.MemoryLocationSet):
            continue
        name = alloc.memorylocations[0].name
        if alloc.kind == "ExternalInput":
            if name != partition_name:
                in_names.append(name)
        elif alloc.kind == "ExternalOutput":
            out_names.append(name)
            shape = tuple(alloc.tensor_shape)
            dtype = mybir.dt.np(alloc.dtype)
            out_avals.append(jax.core.ShapedArray(shape, dtype))
            zero_shapes.append((shape, dtype))
    n_params = len(in_names)
    n_outs = len(out_avals)
    all_in_names = list(in_names) + list(out_names)
    if partition_name is not None:
        all_in_names.append(partition_name)
    donate = tuple(range(n_params, n_params + n_outs))

    def _body(*args):
        operands = list(args)
        if partition_name is not None:
            operands.append(partition_id_tensor())
        outs = _bass_exec_p.bind(
            *operands, out_avals=tuple(out_avals), in_names=tuple(all_in_names),
            out_names=tuple(out_names), lowering_input_output_aliases=(),
            sim_require_finite=True, sim_require_nnan=True, nc=nc)
        return tuple(outs)

    devices = jax.devices()[:n_cores]
    mesh = Mesh(_np.asarray(devices), ("core",))
    in_specs = (PartitionSpec("core"),) * (n_params + n_outs)
    out_specs = (PartitionSpec("core"),) * len(out_names)
    sharded = jax.jit(
        shard_map(_body, mesh=mesh, in_specs=in_specs, out_specs=out_specs,
                  check_rep=False),
        donate_argnums=donate, keep_unused=True)

    def run(in_maps):
        concat_in = [
            np.concatenate([np.asarray(in_maps[c][nm]) for c in range(n_cores)],
                           axis=0)
            for nm in in_names
        ]
        concat_zeros = [np.zeros((n_cores * s[0], *s[1:]), d)
                        for (s, d) in zero_shapes]
        out_arrs = sharded(*concat_in, *concat_zeros)
        return [
            {nm: np.asarray(out_arrs[i]).reshape(n_cores, *out_avals[i].shape)[c]
             for i, nm in enumerate(out_names)}
            for c in range(n_cores)
        ]

    _CACHE[key] = run
    return run


def _build_in_maps(inputs):
    import ml_dtypes

    bfl = ml_dtypes.bfloat16
    emb = np.asarray(inputs["emb"], np.float32)
    tw = np.asarray(inputs["tw"], np.float32)
    w1 = np.asarray(inputs["w1"], np.float32)
    b1v = np.asarray(inputs["b1"], np.float32)
    b2v = np.asarray(inputs["b2"], np.float32)
    w2f = np.asarray(inputs["w2"], np.float32)

    w1bt = np.ascontiguousarray(w1[:, 64:].T).astype(bfl)          # [64, 64]
    w1at = np.ascontiguousarray(0.5 * w1[:, :64].T).astype(bfl)    # [64, 64]
    w2d = np.zeros((128, 32), np.float32)
    w2d[0:64, 0:16] = w2f.T
    w2d[64:128, 16:32] = w2f.T
    w2d = w2d.astype(bfl)
    b1s = np.concatenate([b1v, b1v]).reshape(128, 1).astype(np.float32)
    b2d = np.tile(b2v, 8).reshape(128, 1).astype(np.float32)
    cw = np.zeros((128, 160), bfl)
    cw[0:64, 0:64] = w1bt
    cw[64:128, 0:64] = w1bt
    cw[0:64, 64:128] = w1at
    cw[64:128, 64:128] = w1at
    cw[:, 128:160] = w2d
    cb = np.concatenate([b1s, b2d], axis=1).astype(np.float32)     # [128, 2]

    # host precompute: X2T = (emb * tw).T per batch, interleaved tiles, bf16
    # XU col 2k+w = X2T[k] (both halves); XS top col 2k+w = X2T[(k+1+w)%256],
    # bottom = X2T[(k+65+w)%256]
    B = emb.shape[0]
    X = (emb[:, :L * D] * tw[:L * D]).reshape(B, L, D)     # [B, 256, 64]
    X2T = np.transpose(X, (0, 2, 1)).astype(bfl)           # [B, 64, 256]
    ks = np.arange(256)
    dup = np.repeat(ks, 2)                                 # [512]
    XUh = X2T[:, :, dup]                                   # [B, 64, 512]
    XU = np.concatenate([XUh, XUh], axis=1)                # [B, 128, 512]
    kk = np.arange(324)
    top_idx = ((kk[:, None] + 1 + np.arange(2)[None, :]) % 256).reshape(-1)
    bot_idx = ((kk[:, None] + 65 + np.arange(2)[None, :]) % 256).reshape(-1)
    XS = np.concatenate([X2T[:, :, top_idx], X2T[:, :, bot_idx]], axis=1)

    XU2 = np.concatenate([XU, XU], axis=2)                 # [B, 128, 1024]
    XSsh = np.zeros_like(XS)
    XSsh[:, :, 0:644] = XS[:, :, 4:648]
    XZ = np.concatenate([XU2, XS, XSsh], axis=2)           # [B, 128, 2320]
    shared = {"cw": cw, "cb": cb}
    in_maps = []
    for c in range(N_CORES):
        s = slice(c * B_PER_CORE, (c + 1) * B_PER_CORE)
        m = dict(shared)
        m["xz"] = np.ascontiguousarray(XZ[s])
        in_maps.append(m)
    return in_maps[:N_RUN_CORES]


def kernel(emb, tw, tb, w1, b1, w2, b2, w3, b3, scale):
    run = _get_runner()
    in_maps = _build_in_maps(dict(emb=emb, tw=tw, w1=w1, b1=b1, w2=w2, b2=b2))
    core_results = run(in_maps)

    emb = np.asarray(emb, np.float32)
    tw = np.asarray(tw, np.float32)
    x1 = emb @ tw + float(np.asarray(tb, np.float32)[0])   # [B] host linear

    w3v = np.asarray(w3, np.float32)[0]
    sc = float(np.asarray(scale, np.float32)[0])
    b3f = float(np.asarray(b3, np.float32)[0])
    out = np.zeros(32, np.float32)
    for c in range(N_RUN_CORES):
        acc = core_results[c]["acc_o"]          # [4, 128, ACC_COLS]
        for b in range(B_PER_CORE):
            m = acc[b]
            R = m.reshape(8, 16, ACC_COLS).sum(axis=(0, 2))
            # offset o=128 (it=15 odd cols -> col 5, partitions 64:80 and
            # 96:112) is double counted
            R -= 0.5 * (m[64:80, 5] + m[96:112, 5])
            out[c * B_PER_CORE + b] = (
                x1[c * B_PER_CORE + b] + sc * (R @ w3v + b3f * NPAIRS)
            )
    return out


# revision 16
# speedup vs baseline: 1.0097x; 1.0097x over previous
"""Trainium2 Bass kernel for nn_EpiNN_aaindex (pairwise-MLP GNN reduction).

Math (per batch b):
  x1 = emb@tw + tb                              (computed on host)
  X[i,d] = emb[i*64+d] * tw[i*64+d]             (L=256, D=64; on host)
  s_ij = MLP(concat[(x_i+x_j)/2, |x_i-x_j|])    (64->16->1, LeakyReLU 0.01)
  out_b = x1 + scale * sum_{i<j} s_ij

Strategy: 8 cores, 4 batches/core (data parallel over B=32).
Exact upper-triangle enumeration via cyclic offsets o=1..128:
pairs (i, (i+o) mod 256) for o=1..127 cover each unordered pair once;
o=128 covers each pair twice (0.5 correction on host).

Device inputs per batch (host-precomputed, bf16):
  XU  [128, 512]: both partition halves = [X2T | X2T]  (X2T = X.T [64, 256])
  XSo [128, 320]: top = X2T shifted 1, bottom = shifted 65  (odd offsets)
  XSe [128, 320]: top = X2T shifted 2, bottom = shifted 66  (even offsets)

Main loop: 16 iterations per batch, 8 offsets each, software-pipelined
with a 1-iteration skew so the scalar (ACT) engine never waits on the
H1 -> P2 -> HJ chain:
  iteration u emits: A2(u+1) [DVE sub+abs], P1(u+1) [16 quadrant MMs],
  H1(u+1) [ACT Lrelu], P2(u) [4 col-tiled MMs, dense 8x16 packing],
  HJ [ACT Lrelu+accum, one per 2 iterations, FD=512].

P1 quadrants (64x64 tile mode, 4 concurrent): 4 MMs each =
  w1b@|xi-xj| + 0.5*w1a@xi (XU) + 0.5*w1a@xj (XSx windows).
P2 dense: W2D [128, 32] block-diag(w2.T, w2.T); col-tile j reads
  H1[:, 256j:256j+256], writes P2[32j:32j+32] -> all 128 partitions
  useful (8 offsets x 16 basis), so HJ activation FD is 4x smaller.

Final combine on host: out = x1 + scale*(w3 . R + 32640*b3).
"""
import numpy as np

L, D = 256, 64
B_PER_CORE = 4
N_CORES = 8
NPAIRS = 32640  # 256*255/2
N_ITERS = 16
# quads 0-2 -> cols 0-2; quad 3: its 12-14 -> col 3, it15 even cols -> col 4,
# it15 odd cols (incl. offset o=128) -> col 5
ACC_COLS = 6

_CACHE = {}
import os as _os
N_RUN_CORES = int(_os.environ.get("EPINN_CORES", str(N_CORES)))
HJ_MODE = _os.environ.get("EPINN_HJ", "act")  # act | dve | alt


def _register_custom_ops():
    """Register fused DVE ops via the dve_ops extension point (per-NEFF
    uop table; sha pins harvested at first compile)."""
    import re
    import numpy as np
    from concourse import dve_ops
    from concourse.dve_spec import Spec, Src0, Src1, C0, C1, Zero, maxx
    from operator import add

    if "ops" in _CACHE:
        return _CACHE["ops"]
    existing = {o.name for o in dve_ops.OPS}

    def ref_sub_abs(in0, in1, s0, s1, imm2):
        return np.abs(in0.astype(np.float32) - in1.astype(np.float32))

    def ref_lrelu_acc(in0, in1, s0, s1, imm2):
        z = in0.astype(np.float32) + s0
        b = np.maximum(z, z * s1)
        return b, b.reshape(b.shape[0], -1).sum(axis=-1, keepdims=True)

    specs = {
        "ANT_SUB_ABS": (Spec(body=maxx(Src0 - Src1, Src1 - Src0),
                             reference=ref_sub_abs), {"v3": True, "v4": True}),
        "ANT_LRELU_ACC": (Spec(body=maxx(Src0 + C0, (Src0 + C0) * C1),
                               accum=add, accum_init=Zero,
                               reference=ref_lrelu_acc), {}),
    }
    out = {}
    for name, (spec, perf_en) in specs.items():
        if name in existing:
            out[name] = next(o for o in dve_ops.OPS if o.name == name)
            continue
        row = dve_ops._CUSTOM_DVE_ROW_BASE + len(dve_ops.OPS)
        dve_ops._SUB_OPCODE_FOR_NAME[name] = row
        sha = {}
        op = None
        for _ in range(4):
            op = dve_ops.DveOp(name, spec, False, dict(sha), perf_en)
            try:
                for ver in ("v3", "v4"):
                    op.compile(ver)
                break
            except ValueError as e:
                m = re.search(r"\((v\d+): ([0-9a-f]+)", str(e))
                if not m:
                    raise
                sha[m.group(1)] = m.group(2)
        dve_ops.OPS.append(op)
        out[name] = op
    _CACHE["ops"] = out
    return out


def _build_program():
    import concourse.bacc as bacc
    import concourse.mybir as mybir
    import concourse.tile as tile
    from contextlib import ExitStack

    f32 = mybir.dt.float32
    bf16 = mybir.dt.bfloat16
    u16 = mybir.dt.uint16
    AF = mybir.ActivationFunctionType
    ALU = mybir.AluOpType

    OPS = _register_custom_ops()
    nc = bacc.Bacc("TRN2", target_bir_lowering=False, debug=False,
                   num_devices=N_CORES)

    # ---- DRAM parameters (per core) ----
    xz_d = nc.declare_dram_parameter("xz", [B_PER_CORE, 128, 2320], bf16,
                                     isOutput=False)
    cw_d = nc.declare_dram_parameter("cw", [128, 160], bf16, isOutput=False)
    cb_d = nc.declare_dram_parameter("cb", [128, 2], f32, isOutput=False)

    acc_o = nc.declare_dram_parameter("acc_o", [B_PER_CORE, 128, ACC_COLS],
                                      f32, isOutput=True)

    with tile.TileContext(nc) as tc, ExitStack() as ctx:
        cpool = ctx.enter_context(tc.tile_pool(name="consts", bufs=1))
        xpool = ctx.enter_context(tc.tile_pool(name="xbufs", bufs=2))
        apool = ctx.enter_context(tc.tile_pool(name="abufs", bufs=3))
        hpool = ctx.enter_context(tc.tile_pool(name="hbufs", bufs=3))
        jpool = ctx.enter_context(tc.tile_pool(name="junk", bufs=2))
        opool = ctx.enter_context(tc.tile_pool(name="outs", bufs=2))
        pp1 = ctx.enter_context(tc.tile_pool(name="p1", bufs=2, space="PSUM"))
        pp2 = ctx.enter_context(tc.tile_pool(name="p2", bufs=2, space="PSUM"))
        fold_b2 = HJ_MODE in ("dve", "alt")

        # ---- static weights / consts (2 packed DMAs) ----
        CW = cpool.tile([128, 160], bf16)
        CB = cpool.tile([128, 2], f32)
        nc.sync.dma_start(CW[:], cw_d[:])
        nc.sync.dma_start(CB[:], cb_d[:])
        WB = CW[:, 0:64]
        WA = CW[:, 64:128]
        W2D = CW[:, 128:160]
        B1S = CB[:, 0:1]
        B2D = CB[:, 1:2]

        # ---- per-batch input tiles (loaded by DMA, double buffered) ----
        xt = {}

        def load_batch(b):
            XZ = xpool.tile([128, 2320], bf16, tag="xz")
            nc.sync.dma_start(XZ[:], xz_d[b])
            xt[b] = XZ

        load_batch(0)

        NB = B_PER_CORE
        NU = NB * N_ITERS

        def emit_front(u):
            """A2 build + P1 matmuls + H1 activation for global iter u."""
            b, it = divmod(u, N_ITERS)
            XZ = xt[b]
            XU = XZ[:, 0:512]
            XS = XZ[:, 1024:1672]          # XSi copy0
            XU2v = XZ[:, 0:1024].rearrange("p (g c) -> p g c", g=2, c=512)
            XS2v = XZ[:, 1024:2320].rearrange("p (g c) -> p g c", g=2, c=648)
            c0 = 4 * it

            A2 = apool.tile([128, 1024], bf16, tag="a2")
            # interleaved layout: col 512g + 2k + w = pair (i=k [+128 per
            # later j split], offset 4it+1+2g+w); partition halves: top =
            # base offset, bottom = +64.  XS col 2k+w = X2T[(k+1+w)%256]
            # (top) / X2T[(k+65+w)%256] (bottom); XU col 2k+w = X2T[k].
            A2v = A2[:, :].rearrange("p (g c) -> p g c", g=2, c=512)
            nc.vector._custom_dve(OPS["ANT_SUB_ABS"], out=A2v,
                                  in0=XU2v,
                                  in1=XS2v[:, :, 2 * c0:2 * c0 + 512])

            # P1: 4 concurrent 64x64 quadrants, 3 MMs each; psum col group
            # 0:512 <- window 2c0, 512:1024 <- window 2c0+4 (halves swapped).
            P1 = pp1.tile([128, 1024], f32, tag="p1")
            for (pc, tp, ar) in (
                (0, (0, 0), 0),        # T0
                (0, (64, 64), 64),     # T10
                (512, (64, 0), 64),    # T8  (data rows 64:128 -> psum 0:64)
                (512, (0, 64), 0),     # T2  (data rows 0:64 -> psum 64:128)
            ):
                rg, pr = ar, tp[1]
                cw = 2 * c0 if pc == 0 else 2 * c0 + 4
                ps = P1[pr:pr + 64, pc:pc + 512]
                nc.tensor.matmul(ps, WB[rg:rg + 64, :], A2[rg:rg + 64, pc:pc + 512],
                                 start=True, stop=False, tile_position=tp,
                                 skip_group_check=True)
                nc.tensor.matmul(ps, WA[rg:rg + 64, :], XU[rg:rg + 64, 0:512],
                                 start=False, stop=False, tile_position=tp,
                                 skip_group_check=True)
                nc.tensor.matmul(ps, WA[rg:rg + 64, :],
                                 XS[rg:rg + 64, cw:cw + 512],
                                 start=False, stop=True, tile_position=tp,
                                 skip_group_check=True)

            H1 = hpool.tile([128, 1024], bf16, tag="h1")
            nc.scalar.activation(H1[:], P1[:], AF.Lrelu, bias=B1S[:],
                                 scale=1.0, alpha=0.01)
            return H1

        # ---- software-pipelined main loop ----
        state = {"H1": None, "P2": None, "ACC": None}

        def emit_back(u):
            """P2 matmuls for iter u (+ layer-2 lrelu/sum when quad done)."""
            b, it = divmod(u, N_ITERS)
            if it == 0:
                ACC = opool.tile([128, ACC_COLS], f32, tag="acc", name="ACC")
                nc.gpsimd.memset(ACC[:], 0.0)
                state["ACC"] = ACC
            ACC = state["ACC"]

            ph = it % 4
            q = it // 4
            if ph == 0:
                state["P2"] = pp2.tile([128, 1024], f32, tag="p2", name="P2")
            P2 = state["P2"]
            H1 = state["H1"]
            for j in range(4):
                ps = P2[32 * j:32 * j + 32, 256 * ph:256 * ph + 256]
                nc.tensor.matmul(ps, W2D[:], H1[:, 256 * j:256 * j + 256],
                                 start=True, stop=True,
                                 tile_position=(0, 32 * j),
                                 skip_group_check=True)

            if ph == 3:
                HQ = jpool.tile([128, 1024], bf16, tag="hq", name="HQ")
                LR = OPS["ANT_LRELU_ACC"]
                if it == 15:
                    # o=128 lives in odd cols of phase-3 block (partitions
                    # 64:80 and 96:112) -> isolate in its own accum col
                    HQv = HQ[:, 768:1024].rearrange("p (k w) -> p k w", w=2)
                    P2v = P2[:, 768:1024].rearrange("p (k w) -> p k w", w=2)
                    nc.vector._custom_dve(LR, out=HQ[:, 0:768],
                                          in0=P2[:, 0:768], s0=B2D[:], s1=0.01,
                                          accum_out=ACC[:, 3:4])
                    nc.scalar.activation(HQv[:, :, 0], P2v[:, :, 0], AF.Lrelu,
                                         bias=B2D[:], scale=1.0, alpha=0.01,
                                         accum_out=ACC[:, 4:5])
                    nc.scalar.activation(HQv[:, :, 1], P2v[:, :, 1], AF.Lrelu,
                                         bias=B2D[:], scale=1.0, alpha=0.01,
                                         accum_out=ACC[:, 5:6])
                    nc.sync.dma_start(acc_o[b], ACC[:])
                elif q == 1:
                    nc.scalar.activation(HQ[:], P2[:], AF.Lrelu, bias=B2D[:],
                                         scale=1.0, alpha=0.01,
                                         accum_out=ACC[:, q:q + 1])
                else:
                    nc.vector._custom_dve(LR, out=HQ[:], in0=P2[:],
                                          s0=B2D[:], s1=0.01,
                                          accum_out=ACC[:, q:q + 1])

        state["H1"] = emit_front(0)
        for u in range(NU):
            b, it = divmod(u, N_ITERS)
            if it == 12 and b + 1 < NB:
                load_batch(b + 1)
            H1_next = emit_front(u + 1) if u + 1 < NU else None
            emit_back(u)
            state["H1"] = H1_next

    nc.compile()
    return nc


def _get_program():
    if "prog" not in _CACHE:
        _CACHE["prog"] = _build_program()
    return _CACHE["prog"]


def _get_runner():
    """Build (once) a cached jitted SPMD executable for the program."""
    key = ("runner", N_RUN_CORES)
    if key in _CACHE:
        return _CACHE[key]
    import jax
    import numpy as _np
    import concourse.mybir as mybir
    from jax.sharding import Mesh, PartitionSpec
    from jax.experimental.shard_map import shard_map
    from concourse import bass2jax
    from concourse.bass2jax import _bass_exec_p, partition_id_tensor

    bass2jax.install_neuronx_cc_hook()
    nc = _get_program()
    n_cores = N_RUN_CORES

    partition_name = (nc.partition_id_tensor.name
                      if nc.partition_id_tensor else None)
    in_names, out_names, out_avals, zero_shapes = [], [], [], []
    for alloc in nc.m.functions[0].allocations:
        if not isinstance(alloc, mybir.MemoryLocationSet):
            continue
        name = alloc.memorylocations[0].name
        if alloc.kind == "ExternalInput":
            if name != partition_name:
                in_names.append(name)
        elif alloc.kind == "ExternalOutput":
            out_names.append(name)
            shape = tuple(alloc.tensor_shape)
            dtype = mybir.dt.np(alloc.dtype)
            out_avals.append(jax.core.ShapedArray(shape, dtype))
            zero_shapes.append((shape, dtype))
    n_params = len(in_names)
    n_outs = len(out_avals)
    all_in_names = list(in_names) + list(out_names)
    if partition_name is not None:
        all_in_names.append(partition_name)
    donate = tuple(range(n_params, n_params + n_outs))

    def _body(*args):
        operands = list(args)
        if partition_name is not None:
            operands.append(partition_id_tensor())
        outs = _bass_exec_p.bind(
            *operands, out_avals=tuple(out_avals), in_names=tuple(all_in_names),
            out_names=tuple(out_names), lowering_input_output_aliases=(),
            sim_require_finite=True, sim_require_nnan=True, nc=nc)
        return tuple(outs)

    devices = jax.devices()[:n_cores]
    mesh = Mesh(_np.asarray(devices), ("core",))
    in_specs = (PartitionSpec("core"),) * (n_params + n_outs)
    out_specs = (PartitionSpec("core"),) * len(out_names)
    sharded = jax.jit(
        shard_map(_body, mesh=mesh, in_specs=in_specs, out_specs=out_specs,
                  check_rep=False),
        donate_argnums=donate, keep_unused=True)

    def run(in_maps):
        concat_in = [
            np.concatenate([np.asarray(in_maps[c][nm]) for c in range(n_cores)],
                           axis=0)
            for nm in in_names
        ]
        concat_zeros = [np.zeros((n_cores * s[0], *s[1:]), d)
                        for (s, d) in zero_shapes]
        out_arrs = sharded(*concat_in, *concat_zeros)
        return [
            {nm: np.asarray(out_arrs[i]).reshape(n_cores, *out_avals[i].shape)[c]
             for i, nm in enumerate(out_names)}
            for c in range(n_cores)
        ]

    _CACHE[key] = run
    return run


def _build_in_maps(inputs):
    import ml_dtypes

    bfl = ml_dtypes.bfloat16
    emb = np.asarray(inputs["emb"], np.float32)
    tw = np.asarray(inputs["tw"], np.float32)
    w1 = np.asarray(inputs["w1"], np.float32)
    b1v = np.asarray(inputs["b1"], np.float32)
    b2v = np.asarray(inputs["b2"], np.float32)
    w2f = np.asarray(inputs["w2"], np.float32)

    w1bt = np.ascontiguousarray(w1[:, 64:].T).astype(bfl)          # [64, 64]
    w1at = np.ascontiguousarray(0.5 * w1[:, :64].T).astype(bfl)    # [64, 64]
    w2d = np.zeros((128, 32), np.float32)
    w2d[0:64, 0:16] = w2f.T
    w2d[64:128, 16:32] = w2f.T
    w2d = w2d.astype(bfl)
    b1s = np.concatenate([b1v, b1v]).reshape(128, 1).astype(np.float32)
    b2d = np.tile(b2v, 8).reshape(128, 1).astype(np.float32)
    cw = np.zeros((128, 160), bfl)
    cw[0:64, 0:64] = w1bt
    cw[64:128, 0:64] = w1bt
    cw[0:64, 64:128] = w1at
    cw[64:128, 64:128] = w1at
    cw[:, 128:160] = w2d
    cb = np.concatenate([b1s, b2d], axis=1).astype(np.float32)     # [128, 2]

    # host precompute: X2T = (emb * tw).T per batch, interleaved tiles, bf16
    # XU col 2k+w = X2T[k] (both halves); XS top col 2k+w = X2T[(k+1+w)%256],
    # bottom = X2T[(k+65+w)%256]
    B = emb.shape[0]
    X = (emb[:, :L * D] * tw[:L * D]).reshape(B, L, D)     # [B, 256, 64]
    X2T = np.transpose(X, (0, 2, 1)).astype(bfl)           # [B, 64, 256]
    ks = np.arange(256)
    dup = np.repeat(ks, 2)                                 # [512]
    XUh = X2T[:, :, dup]                                   # [B, 64, 512]
    XU = np.concatenate([XUh, XUh], axis=1)                # [B, 128, 512]
    kk = np.arange(324)
    top_idx = ((kk[:, None] + 1 + np.arange(2)[None, :]) % 256).reshape(-1)
    bot_idx = ((kk[:, None] + 65 + np.arange(2)[None, :]) % 256).reshape(-1)
    XS = np.concatenate([X2T[:, :, top_idx], X2T[:, :, bot_idx]], axis=1)

    XU2 = np.concatenate([XU, XU], axis=2)                 # [B, 128, 1024]
    XSsh = np.zeros_like(XS)
    XSsh[:, :, 0:644] = XS[:, :, 4:648]
    XZ = np.concatenate([XU2, XS, XSsh], axis=2)           # [B, 128, 2320]
    shared = {"cw": cw, "cb": cb}
    in_maps = []
    for c in range(N_CORES):
        s = slice(c * B_PER_CORE, (c + 1) * B_PER_CORE)
        m = dict(shared)
        m["xz"] = np.ascontiguousarray(XZ[s])
        in_maps.append(m)
    return in_maps[:N_RUN_CORES]


def kernel(emb, tw, tb, w1, b1, w2, b2, w3, b3, scale):
    run = _get_runner()
    in_maps = _build_in_maps(dict(emb=emb, tw=tw, w1=w1, b1=b1, w2=w2, b2=b2))
    core_results = run(in_maps)

    emb = np.asarray(emb, np.float32)
    tw = np.asarray(tw, np.float32)
    x1 = emb @ tw + float(np.asarray(tb, np.float32)[0])   # [B] host linear

    w3v = np.asarray(w3, np.float32)[0]
    sc = float(np.asarray(scale, np.float32)[0])
    b3f = float(np.asarray(b3, np.float32)[0])
    out = np.zeros(32, np.float32)
    for c in range(N_RUN_CORES):
        acc = core_results[c]["acc_o"]          # [4, 128, ACC_COLS]
        for b in range(B_PER_CORE):
            m = acc[b]
            R = m.reshape(8, 16, ACC_COLS).sum(axis=(0, 2))
            # offset o=128 (it=15 odd cols -> col 5, partitions 64:80 and
            # 96:112) is double counted
            R -= 0.5 * (m[64:80, 5] + m[96:112, 5])
            out[c * B_PER_CORE + b] = (
                x1[c * B_PER_CORE + b] + sc * (R @ w3v + b3f * NPAIRS)
            )
    return out
